# revision 1
# baseline (speedup 1.0000x reference)
"""Trainium2 Bass kernel for nn_AwesomeGRU (SEQ=512, B=64, DIM=1024, UNITS=1024).

Algorithm: the `reset` input zeroes h *before* each masked step, so each batch
row's recurrence splits into independent segments (h carries over only within
a segment). Classic packed-sequence reformulation:

  host: enumerate segments, sort by length desc, deal round-robin to 8 cores,
        lay tokens out depth-major ((depth, segment-rank) order). Pass j
        processes all tokens at depth j — a contiguous row block whose h
        inputs are a PREFIX of pass j-1's outputs (no gather).
  core: for each pass j: PSUM <- x_j @ W_ih^T (+ h_j @ W_hh^T if j>0), then
        gates elementwise, h_out -> DRAM (it IS the output) + fp16 copy in
        SBUF for pass j+1's matmul.
  host: inverse-permute output tokens to (seq, b, units).

Everything is feature-major on device: activations stored (units, rows) so
no transposes are ever needed. Matmul operands fp16 (same PE rate as bf16 on
TRN2, 3 more mantissa bits; PSUM accumulates fp32), elementwise fp32. Depth-0
tokens (h=0) skip the h-matmul exactly.

Self-contained: derives everything from the runtime value of `reset`.
"""
import os
import numpy as np

import concourse.bacc as bacc
import concourse.mybir as mybir
import concourse.tile as tile
from concourse.bass_utils import run_bass_kernel_spmd

SEQ, B, DIM, UNITS = 512, 64, 1024, 1024
NCORES = 8
P = 128
CG = DIM // P        # 8 contraction groups per matmul side
UG = UNITS // P      # 8 unit groups
CH = 512             # row-chunk (free dim / PSUM bank)
dt = mybir.dt
f32 = dt.float32
bf16 = dt.float16  # fp16: same PE rate as bf16, 3 more mantissa bits

LAST_EXEC_NS = None  # set when GRU_TRACE=1


# ---------------------------------------------------------------- host plan

def _build_plan(reset_sb, h0_any):
    """reset_sb: (SEQ, B) bool. Returns (m_j schedule, per-core token maps).

    Segment starts: t=0 always (h0 seed row: h0[b] unless reset[0,b]), and
    every t>0 with reset=1 (h zeroed exactly).
    """
    segs = []  # (length, b, t_start)
    for b in range(B):
        col = reset_sb[:, b]
        starts = [0] + [t for t in range(1, SEQ) if col[t]]
        for i, s in enumerate(starts):
            e = starts[i + 1] if i + 1 < len(starts) else SEQ
            segs.append((e - s, b, s))
    segs.sort(key=lambda x: (-x[0], x[1], x[2]))
    Lmax = segs[0][0]
    n_j = [0] * Lmax
    for L, _, _ in segs:
        for j in range(L):
            n_j[j] += 1
    m_j = [(n + NCORES - 1) // NCORES for n in n_j]

    plans = []
    for c in range(NCORES):
        mysegs = segs[c::NCORES]
        tok = np.full(sum(m_j), -1, np.int64)  # flat t*B+b index or -1 pad
        seed_b = np.full(m_j[0], -1, np.int64)  # batch row for h seed (pass 0)
        off = 0
        for j in range(Lmax):
            for r in range(m_j[j]):
                if r < len(mysegs) and mysegs[r][0] > j:
                    L, b, s = mysegs[r]
                    tok[off + r] = (s + j) * B + b
                    if j == 0 and s == 0 and h0_any and not reset_sb[0, b]:
                        seed_b[r] = b
            off += m_j[j]
        plans.append((tok, seed_b))
    return m_j, plans


# ------------------------------------------------------------- device build

def _chunks(m):
    """Split m rows into balanced chunks of <= CH."""
    nch = (m + CH - 1) // CH
    base, rem = divmod(m, nch)
    out, off = [], 0
    for i in range(nch):
        f = base + (1 if i < rem else 0)
        out.append((off, f))
        off += f
    return out


def _build_nc(m_j, use_seed, j_pre):
    """j_pre: first pass whose gi comes from the fp16 presweep buffer."""
    Lmax = len(m_j)
    N_pad = sum(m_j)
    M_off = np.cumsum([0] + m_j)  # row offset of each pass block
    R0 = int(M_off[j_pre]) if j_pre < Lmax else N_pad  # presweep row range
    RN = N_pad - R0

    nc = bacc.Bacc("TRN2", target_bir_lowering=False, debug=False,
                   num_devices=NCORES)
    xT = nc.dram_tensor("xT", [DIM, N_pad], bf16, kind="ExternalInput")
    wihT = nc.dram_tensor("wihT", [DIM, 3 * UNITS], bf16, kind="ExternalInput")
    whhT = nc.dram_tensor("whhT", [UNITS, 3 * UNITS], bf16, kind="ExternalInput")
    biases = nc.dram_tensor("biases", [UNITS, 4], f32, kind="ExternalInput")
    outT = nc.dram_tensor("outT", [UNITS, N_pad], f32, kind="ExternalOutput")
    hseedT = None
    if use_seed:
        hseedT = nc.dram_tensor("hseedT", [UNITS, m_j[0]], bf16,
                                kind="ExternalInput")

    Sig = mybir.ActivationFunctionType.Sigmoid
    Tanh = mybir.ActivationFunctionType.Tanh
    ADD = mybir.AluOpType.add
    MULT = mybir.AluOpType.mult

    with tile.TileContext(nc) as tc:
        with (
            tc.tile_pool(name="wpool", bufs=1) as wpool,
            tc.tile_pool(name="xpool", bufs=2) as xpool,
            tc.tile_pool(name="hpool", bufs=2) as hpool,
            tc.tile_pool(name="spool", bufs=2) as spool,
            tc.tile_pool(name="ppool", bufs=2, space="PSUM") as ppool,
        ):
            wih_t = wpool.tile([P, CG, 3 * UNITS], bf16, tag="wih")
            whh_t = wpool.tile([P, CG, 3 * UNITS], bf16, tag="whh")

            x_tiles = {}

            def get_x_tile(jj, ooff, ff):
                key = (jj, ooff)
                if key not in x_tiles:
                    x_t = xpool.tile([P, CG, CH], bf16, tag="x", name="x_t")
                    bb = int(M_off[jj]) + ooff
                    for c in range(CG):
                        nc.sync.dma_start(out=x_t[:, c, :ff],
                                          in_=xT[c * P:(c + 1) * P, bb: bb + ff])
                    x_tiles[key] = x_t
                return x_tiles[key]

            # DMA emission order = need order: r-gate weights, first x chunk,
            # remaining W_ih gates + biases, second x chunk. W_hh and the
            # presweep are emitted later (needed from pass 1 / pass j_pre).
            for c in range(CG):
                nc.sync.dma_start(out=wih_t[:, c, 0:UNITS],
                                  in_=wihT[c * P:(c + 1) * P, 0:UNITS])
            ch0 = _chunks(m_j[0])
            get_x_tile(0, *ch0[0])
            for g in (1, 2):
                for c in range(CG):
                    nc.sync.dma_start(
                        out=wih_t[:, c, g * UNITS:(g + 1) * UNITS],
                        in_=wihT[c * P:(c + 1) * P, g * UNITS:(g + 1) * UNITS])
            b_t = wpool.tile([P, UG, 4], f32, tag="bias")
            for g in range(UG):
                nc.sync.dma_start(out=b_t[:, g, :], in_=biases[g * P:(g + 1) * P, :])
            if len(ch0) > 1:
                get_x_tile(0, *ch0[1])

            def emit_whh():
                for g in range(3):
                    for c in range(CG):
                        nc.sync.dma_start(
                            out=whh_t[:, c, g * UNITS:(g + 1) * UNITS],
                            in_=whhT[c * P:(c + 1) * P, g * UNITS:(g + 1) * UNITS])

            gi_pre = (wpool.tile([P, 3 * UG, RN], dt.float16, tag="gi_pre",
                                name="gi_pre")
                      if RN > 0 else None)

            def emit_presweep():
                # gi for all deep-pass rows in one efficient batched matmul
                with nc.named_scope("presweep"):
                    xp_t = xpool.tile([P, CG, RN], bf16, tag="xpre", bufs=1, name="xp_t")
                    for c in range(CG):
                        nc.sync.dma_start(out=xp_t[:, c, :],
                                          in_=xT[c * P:(c + 1) * P, R0:N_pad])
                    for gu in range(3 * UG):
                        ps_p = ppool.tile([P, CH], f32, tag="ps_gin",
                                          name="ps_pre")
                        for c in range(CG):
                            nc.tensor.matmul(
                                ps_p[:, :RN],
                                lhsT=wih_t[:, c, gu * P:(gu + 1) * P],
                                rhs=xp_t[:, c, :],
                                start=(c == 0), stop=(c == CG - 1))
                        nc.vector.tensor_copy(gi_pre[:, gu, :], ps_p[:, :RN])

            if use_seed:
                emit_whh()  # pass 0 already needs W_hh

            h_cur = None  # bf16 SBUF (P, CG, m_j[j]) input h for current pass
            for j in range(Lmax):
                if j == j_pre and gi_pre is not None:
                    emit_presweep()
                scope = nc.named_scope(f"pass{j:02d}")
                scope.__enter__()
                m = m_j[j]
                m_next = m_j[j + 1] if j + 1 < Lmax else 0
                has_h = (j > 0) or use_seed
                pre = j >= j_pre
                base = int(M_off[j])
                h_next = (hpool.tile([P, CG, m_next], bf16, tag="hbuf",
                                     name=f"hbuf{j}")
                          if m_next > 0 else None)

                for ci, (off, f) in enumerate(_chunks(m)):
                    if not pre:
                        x_t = get_x_tile(j, off, f)
                    if j == 0 and use_seed:
                        hs_t = xpool.tile([P, CG, CH], bf16, tag="hseed", name="hs_t", bufs=1)
                        for c in range(CG):
                            nc.sync.dma_start(
                                out=hs_t[:, c, :f],
                                in_=hseedT[c * P:(c + 1) * P, off: off + f])
                        h_in = lambda c: hs_t[:, c, :f]
                    elif has_h:
                        h_in = lambda c: h_cur[:, c, off: off + f]
                    else:
                        h_in = None
                    # presweep-relative row slice for this chunk
                    p0 = base + off - R0

                    def x_mms(ps, gate, stop_at_end):
                        for c in range(CG):
                            nc.tensor.matmul(
                                ps[:, :f],
                                lhsT=wih_t[:, c, gate * UNITS + u * P:
                                           gate * UNITS + (u + 1) * P],
                                rhs=x_t[:, c, :f],
                                start=(c == 0),
                                stop=(stop_at_end and c == CG - 1))

                    def h_mms(ps, gate, cs, do_start, do_stop):
                        cs = list(cs)
                        for c in cs:
                            nc.tensor.matmul(
                                ps[:, :f],
                                lhsT=whh_t[:, c, gate * UNITS + u * P:
                                           gate * UNITS + (u + 1) * P],
                                rhs=h_in(c),
                                start=(do_start and c == cs[0]),
                                stop=(do_stop and c == cs[-1]),
                                skip_group_check=True)

                    for u in range(UG):
                        ps_r = ppool.tile([P, CH], f32, tag="ps_r")
                        ps_z = ppool.tile([P, CH], f32, tag="ps_z")
                        if not pre:
                            ps_gin = ppool.tile([P, CH], f32, tag="ps_gin")
                        ps_ghn = (ppool.tile([P, CH], f32, tag="ps_ghn",
                                             name="ps_ghn")
                                  if has_h else None)

                        # For the first unit-tile of a chunk, defer every
                        # gate's c=7 h-matmul to the end: it waits on the
                        # previous pass's last h cast, and deferring lets the
                        # other 21+ matmuls run during that wait.
                        split = has_h and u == 0 and off == 0
                        early = range(CG - 1) if split else range(CG)
                        if not pre:
                            x_mms(ps_r, 0, stop_at_end=not has_h)
                            if has_h:
                                h_mms(ps_r, 0, early, False, not split)
                            x_mms(ps_z, 1, stop_at_end=not has_h)
                            if has_h:
                                h_mms(ps_z, 1, early, False, not split)
                            x_mms(ps_gin, 2, stop_at_end=True)
                            if has_h:
                                h_mms(ps_ghn, 2, early, True, not split)
                        else:
                            h_mms(ps_r, 0, early, True, not split)
                            h_mms(ps_z, 1, early, True, not split)
                            h_mms(ps_ghn, 2, early, True, not split)
                        if split:
                            h_mms(ps_r, 0, [CG - 1], False, True)
                            h_mms(ps_z, 1, [CG - 1], False, True)
                            h_mms(ps_ghn, 2, [CG - 1], False, True)

                        r_sb = spool.tile([P, CH], f32, tag="r")
                        z_sb = spool.tile([P, CH], f32, tag="z")
                        n_sb = spool.tile([P, CH], f32, tag="n")
                        h_sb = spool.tile([P, CH], f32, tag="r" if use_seed else "h",
                                          name="h_sb")
                        t2 = spool.tile([P, CH], f32, tag="t2")
                        if pre:
                            # r = sig((ps_r + b_r) + gi_r) ; same for z
                            nc.vector.scalar_tensor_tensor(
                                r_sb[:, :f], ps_r[:, :f], b_t[:, u, 0:1],
                                gi_pre[:, u, p0:p0 + f], op0=ADD, op1=ADD)
                            nc.scalar.activation(r_sb[:, :f], r_sb[:, :f], Sig)
                            nc.vector.scalar_tensor_tensor(
                                z_sb[:, :f], ps_z[:, :f], b_t[:, u, 1:2],
                                gi_pre[:, UG + u, p0:p0 + f], op0=ADD, op1=ADD)
                            nc.scalar.activation(z_sb[:, :f], z_sb[:, :f], Sig)
                            nc.vector.scalar_tensor_tensor(
                                t2[:, :f], ps_ghn[:, :f], b_t[:, u, 3:4],
                                r_sb[:, :f], op0=ADD, op1=MULT)
                            arg = spool.tile([P, CH], f32, tag="d", name="arg")
                            nc.vector.tensor_add(arg[:, :f], t2[:, :f],
                                                 gi_pre[:, 2 * UG + u, p0:p0 + f])
                            nc.scalar.activation(n_sb[:, :f], arg[:, :f], Tanh,
                                                 bias=b_t[:, u, 2:3])
                        else:
                            nc.scalar.activation(r_sb[:, :f], ps_r[:, :f], Sig,
                                                 bias=b_t[:, u, 0:1])
                            nc.scalar.activation(z_sb[:, :f], ps_z[:, :f], Sig,
                                                 bias=b_t[:, u, 1:2])
                            if has_h:
                                # t2 = (ps_ghn + b_hhn) * r
                                nc.vector.scalar_tensor_tensor(
                                    t2[:, :f], ps_ghn[:, :f], b_t[:, u, 3:4],
                                    r_sb[:, :f], op0=ADD, op1=MULT)
                                arg = spool.tile([P, CH], f32, tag="d", name="arg")
                                nc.vector.tensor_add(arg[:, :f], t2[:, :f],
                                                     ps_gin[:, :f])
                                nc.scalar.activation(n_sb[:, :f], arg[:, :f],
                                                     Tanh, bias=b_t[:, u, 2:3])
                            else:
                                # t2 = r*b_hhn + ps_gin ; n = tanh(t2 + b_ihn)
                                nc.vector.scalar_tensor_tensor(
                                    t2[:, :f], r_sb[:, :f], b_t[:, u, 3:4],
                                    ps_gin[:, :f], op0=MULT, op1=ADD)
                                nc.scalar.activation(n_sb[:, :f], t2[:, :f],
                                                     Tanh, bias=b_t[:, u, 2:3])
                        if has_h:
                            # h = n + z*(h_prev - n)   (h_prev via bf16 tile)
                            d_sb = spool.tile([P, CH], f32, tag="d")
                            nc.vector.tensor_sub(d_sb[:, :f], h_in(u), n_sb[:, :f])
                            zd = spool.tile([P, CH], f32, tag="t2", name="zd")
                            nc.vector.tensor_mul(zd[:, :f], z_sb[:, :f], d_sb[:, :f])
                            nc.vector.tensor_add(h_sb[:, :f], n_sb[:, :f], zd[:, :f])
                        else:
                            # h = (1-z)*n = n - z*n
                            zd = spool.tile([P, CH], f32, tag="t2", name="zd")
                            nc.vector.tensor_mul(zd[:, :f], z_sb[:, :f], n_sb[:, :f])
                            nc.vector.tensor_sub(h_sb[:, :f], n_sb[:, :f], zd[:, :f])

                        nc.sync.dma_start(
                            out=outT[u * P:(u + 1) * P, base + off: base + off + f],
                            in_=h_sb[:, :f])
                        pf = min(m_next - off, f)
                        if pf > 0:
                            nc.vector.tensor_copy(h_next[:, u, off: off + pf],
                                                  h_sb[:, :pf])
                    if j == 0 and ci == 0 and not use_seed:
                        emit_whh()  # W_hh drains during pass-0 compute
                    if not pre and (j, off) in x_tiles:
                        del x_tiles[(j, off)]  # consumed; let the slot recycle
                h_cur = h_next
                scope.__exit__(None, None, None)
    nc.compile()
    return nc


# ------------------------------------------------------------------- kernel

def kernel(x, h0, reset, W_ih, W_hh, b_ih, b_hh):
    global LAST_EXEC_NS
    x = np.asarray(x, np.float32)
    h0 = np.asarray(h0, np.float32)
    reset_sb = np.asarray(reset).reshape(SEQ, B).astype(bool)
    W_ih = np.asarray(W_ih, np.float32)
    W_hh = np.asarray(W_hh, np.float32)
    b_ih = np.asarray(b_ih, np.float32)
    b_hh = np.asarray(b_hh, np.float32)

    h0_any = bool(np.any(h0))
    m_j, plans = _build_plan(reset_sb, h0_any)
    N_pad = sum(m_j)

    b_sum = b_ih + b_hh
    biases = np.stack([b_sum[:UNITS], b_sum[UNITS:2 * UNITS],
                       b_ih[2 * UNITS:], b_hh[2 * UNITS:]], axis=1)
    biases = np.ascontiguousarray(biases, np.float32)
    wihT = np.ascontiguousarray(W_ih.T).astype(np.float16)
    whhT = np.ascontiguousarray(W_hh.T).astype(np.float16)

    xf = x.reshape(SEQ * B, DIM)
    in_maps = []
    for c in range(NCORES):
        tok, seed_b = plans[c]
        real = tok >= 0
        xg = np.zeros((N_pad, DIM), np.float32)
        xg[real] = xf[tok[real]]
        m = {
            "xT": np.ascontiguousarray(xg.T).astype(np.float16),
            "wihT": wihT, "whhT": whhT, "biases": biases,
        }
        if h0_any:
            hs = np.zeros((m_j[0], UNITS), np.float32)
            sreal = seed_b >= 0
            hs[sreal] = h0[seed_b[sreal]]
            m["hseedT"] = np.ascontiguousarray(hs.T).astype(np.float16)
        in_maps.append(m)

    j_pre = 1
    while j_pre < len(m_j) and sum(m_j[j_pre:]) > CH:
        j_pre += 1
    nc = _build_nc(m_j, use_seed=h0_any, j_pre=j_pre)
    trace = os.environ.get("GRU_TRACE", "0") == "1"
    res = run_bass_kernel_spmd(nc, in_maps, list(range(NCORES)), trace=trace)
    LAST_EXEC_NS = res.exec_time_ns

    out = np.zeros((SEQ * B, UNITS), np.float32)
    for c in range(NCORES):
        tok, _ = plans[c]
        real = tok >= 0
        out[tok[real]] = res.results[c]["outT"].T[real]
    return out.reshape(SEQ, B, UNITS)



# revision 6
# speedup vs baseline: 1.0547x; 1.0547x over previous
"""Trainium2 Bass kernel for nn_AwesomeGRU (SEQ=512, B=64, DIM=1024, UNITS=1024).

Algorithm: the `reset` input zeroes h *before* each masked step, so each batch
row's recurrence splits into independent segments (h carries over only within
a segment). Classic packed-sequence reformulation:

  host: enumerate segments, sort by length desc, deal round-robin to 8 cores,
        lay tokens out depth-major ((depth, segment-rank) order). Pass j
        processes all tokens at depth j — a contiguous row block whose h
        inputs are a PREFIX of pass j-1's outputs (no gather).
  core: for each pass j: PSUM <- x_j @ W_ih^T (+ h_j @ W_hh^T if j>0), then
        gates elementwise, h_out -> DRAM (it IS the output) + fp16 copy in
        SBUF for pass j+1's matmul.
  host: inverse-permute output tokens to (seq, b, units).

Everything is feature-major on device: activations stored (units, rows) so
no transposes are ever needed. Matmul operands fp16 (same PE rate as bf16 on
TRN2, 3 more mantissa bits; PSUM accumulates fp32), elementwise fp32. Depth-0
tokens (h=0) skip the h-matmul exactly.

Self-contained: derives everything from the runtime value of `reset`.
"""
import os
import numpy as np
import ml_dtypes

import concourse.bacc as bacc
import concourse.mybir as mybir
import concourse.tile as tile
from concourse.bass_utils import run_bass_kernel_spmd

SEQ, B, DIM, UNITS = 512, 64, 1024, 1024
NCORES = 8
P = 128
CG = DIM // P        # 8 contraction groups per matmul side
PG = CG // 2         # 4 fp8 DoubleRow pair-groups (2x128 contraction each)
UG = UNITS // P      # 8 unit groups
CH = 512             # row-chunk (free dim / PSUM bank)
dt = mybir.dt
f32 = dt.float32
bf16 = dt.float16  # fp16: same PE rate as bf16, 3 more mantissa bits
fp8 = dt.float8e4  # e4m3: r-gate matmuls in DoubleRow mode (2x PE rate)
W8SCALE = 4096.0   # fp8 weights stored *4096 (else all subnormal); ps_r scaled

LAST_EXEC_NS = None  # set when GRU_TRACE=1


# ---------------------------------------------------------------- host plan

def _build_plan(reset_sb, h0_any):
    """reset_sb: (SEQ, B) bool. Returns (m_j schedule, per-core token maps).

    Segment starts: t=0 always (h0 seed row: h0[b] unless reset[0,b]), and
    every t>0 with reset=1 (h zeroed exactly).
    """
    segs = []  # (length, b, t_start)
    for b in range(B):
        col = reset_sb[:, b]
        starts = [0] + [t for t in range(1, SEQ) if col[t]]
        for i, s in enumerate(starts):
            e = starts[i + 1] if i + 1 < len(starts) else SEQ
            segs.append((e - s, b, s))
    segs.sort(key=lambda x: (-x[0], x[1], x[2]))
    Lmax = segs[0][0]
    n_j = [0] * Lmax
    for L, _, _ in segs:
        for j in range(L):
            n_j[j] += 1
    m_j = [(n + NCORES - 1) // NCORES for n in n_j]

    plans = []
    for c in range(NCORES):
        mysegs = segs[c::NCORES]
        tok = np.full(sum(m_j), -1, np.int64)  # flat t*B+b index or -1 pad
        seed_b = np.full(m_j[0], -1, np.int64)  # batch row for h seed (pass 0)
        off = 0
        for j in range(Lmax):
            for r in range(m_j[j]):
                if r < len(mysegs) and mysegs[r][0] > j:
                    L, b, s = mysegs[r]
                    tok[off + r] = (s + j) * B + b
                    if j == 0 and s == 0 and h0_any and not reset_sb[0, b]:
                        seed_b[r] = b
            off += m_j[j]
        plans.append((tok, seed_b))
    return m_j, plans


# ------------------------------------------------------------- device build

def _chunks(m):
    """Split m rows into balanced chunks of <= CH."""
    nch = (m + CH - 1) // CH
    base, rem = divmod(m, nch)
    out, off = [], 0
    for i in range(nch):
        f = base + (1 if i < rem else 0)
        out.append((off, f))
        off += f
    return out


def _build_nc(m_j, use_seed, j_pre):
    """j_pre: first pass whose gi comes from the fp16 presweep buffer."""
    Lmax = len(m_j)
    N_pad = sum(m_j)
    M_off = np.cumsum([0] + m_j)  # row offset of each pass block
    R0 = int(M_off[j_pre]) if j_pre < Lmax else N_pad  # presweep row range
    RN = N_pad - R0

    nc = bacc.Bacc("TRN2", target_bir_lowering=False, debug=False,
                   num_devices=NCORES)
    xT = nc.dram_tensor("xT", [DIM, N_pad], bf16, kind="ExternalInput")
    xT8 = nc.dram_tensor("xT8", [DIM, N_pad], fp8, kind="ExternalInput")
    # fp16 weights hold only the z and n gates; r is fp8 (DoubleRow)
    wihT = nc.dram_tensor("wihT", [DIM, 2 * UNITS], bf16, kind="ExternalInput")
    whhT = nc.dram_tensor("whhT", [UNITS, 2 * UNITS], bf16, kind="ExternalInput")
    wih8T = nc.dram_tensor("wih8T", [DIM, UNITS], fp8, kind="ExternalInput")
    whh8T = nc.dram_tensor("whh8T", [UNITS, UNITS], fp8, kind="ExternalInput")
    biases = nc.dram_tensor("biases", [UNITS, 4], f32, kind="ExternalInput")
    outT = nc.dram_tensor("outT", [UNITS, N_pad], bf16, kind="ExternalOutput")
    hseedT = hseed8T = None
    if use_seed:
        hseedT = nc.dram_tensor("hseedT", [UNITS, m_j[0]], bf16,
                                kind="ExternalInput")
        hseed8T = nc.dram_tensor("hseed8T", [UNITS, m_j[0]], fp8,
                                 kind="ExternalInput")

    Sig = mybir.ActivationFunctionType.Sigmoid
    Tanh = mybir.ActivationFunctionType.Tanh
    ADD = mybir.AluOpType.add
    MULT = mybir.AluOpType.mult
    DR = mybir.MatmulPerfMode.DoubleRow
    RS = 1.0 / W8SCALE

    with tile.TileContext(nc) as tc:
        with (
            tc.tile_pool(name="wpool", bufs=1) as wpool,
            tc.tile_pool(name="xpool", bufs=2) as xpool,
            tc.tile_pool(name="hpool", bufs=2) as hpool,
            tc.tile_pool(name="spool", bufs=2) as spool,
            tc.tile_pool(name="ppool", bufs=2, space="PSUM") as ppool,
        ):
            wih_t = wpool.tile([P, CG, 2 * UNITS], bf16, tag="wih")
            whh_t = wpool.tile([P, CG, 2 * UNITS], bf16, tag="whh")
            wih8_t = wpool.tile([P, PG, 2, UNITS], fp8, tag="wih8")
            whh8_t = wpool.tile([P, PG, 2, UNITS], fp8, tag="whh8")

            x_tiles = {}
            x8_tiles = {}

            def get_x_tile(jj, ooff, ff):
                key = (jj, ooff)
                if key not in x_tiles:
                    x_t = xpool.tile([P, CG, CH], bf16, tag="x", name="x_t")
                    bb = int(M_off[jj]) + ooff
                    for c in range(CG):
                        nc.sync.dma_start(out=x_t[:, c, :ff],
                                          in_=xT[c * P:(c + 1) * P, bb: bb + ff])
                    x_tiles[key] = x_t
                return x_tiles[key]

            def get_x8_tile(jj, ooff, ff):
                key = (jj, ooff)
                if key not in x8_tiles:
                    x_t = xpool.tile([P, PG, 2, CH], fp8, tag="x8", name="x8_t")
                    bb = int(M_off[jj]) + ooff
                    for g in range(PG):
                        for i in range(2):
                            c = 2 * g + i
                            nc.sync.dma_start(
                                out=x_t[:, g, i, :ff],
                                in_=xT8[c * P:(c + 1) * P, bb: bb + ff])
                    x8_tiles[key] = x_t
                return x8_tiles[key]

            # DMA emission order = need order: r-gate fp8 weights, first x
            # chunks, z/n fp16 W_ih + biases, second x chunk. W_hh and the
            # presweep are emitted later (needed from pass 1 / pass j_pre).
            for g in range(PG):
                for i in range(2):
                    c = 2 * g + i
                    nc.sync.dma_start(out=wih8_t[:, g, i, :],
                                      in_=wih8T[c * P:(c + 1) * P, :])
            ch0 = _chunks(m_j[0])
            get_x8_tile(0, *ch0[0])
            for c in range(CG):
                nc.sync.dma_start(out=wih_t[:, c, 0:UNITS],
                                  in_=wihT[c * P:(c + 1) * P, 0:UNITS])
            get_x_tile(0, *ch0[0])
            for c in range(CG):
                nc.sync.dma_start(
                    out=wih_t[:, c, UNITS:2 * UNITS],
                    in_=wihT[c * P:(c + 1) * P, UNITS:2 * UNITS])
            b_t = wpool.tile([P, UG, 4], f32, tag="bias")
            for g in range(UG):
                nc.sync.dma_start(out=b_t[:, g, :], in_=biases[g * P:(g + 1) * P, :])
            if len(ch0) > 1:
                get_x8_tile(0, *ch0[1])
                get_x_tile(0, *ch0[1])

            def emit_whh():
                for g in range(PG):
                    for i in range(2):
                        c = 2 * g + i
                        nc.sync.dma_start(out=whh8_t[:, g, i, :],
                                          in_=whh8T[c * P:(c + 1) * P, :])
                for g in range(2):
                    for c in range(CG):
                        nc.sync.dma_start(
                            out=whh_t[:, c, g * UNITS:(g + 1) * UNITS],
                            in_=whhT[c * P:(c + 1) * P, g * UNITS:(g + 1) * UNITS])

            gi_pre = (wpool.tile([P, 3 * UG, RN], dt.float16, tag="gi_pre",
                                name="gi_pre")
                      if RN > 0 else None)

            def emit_presweep():
                # gi for all deep-pass rows in one efficient batched matmul.
                # r-gate groups (gu 0..UG-1) in fp8 DoubleRow, stored *W8SCALE.
                with nc.named_scope("presweep"):
                    xp8_t = xpool.tile([P, PG, 2, RN], fp8, tag="xpre8",
                                       bufs=1, name="xp8_t")
                    for g in range(PG):
                        for i in range(2):
                            c = 2 * g + i
                            nc.sync.dma_start(out=xp8_t[:, g, i, :],
                                              in_=xT8[c * P:(c + 1) * P, R0:N_pad])
                    xp_t = xpool.tile([P, CG, RN], bf16, tag="xpre", bufs=1, name="xp_t")
                    for c in range(CG):
                        nc.sync.dma_start(out=xp_t[:, c, :],
                                          in_=xT[c * P:(c + 1) * P, R0:N_pad])
                    for gu in range(3 * UG):
                        ps_p = ppool.tile([P, CH], f32, tag="ps_gin",
                                          name="ps_pre")
                        if gu < UG:  # r gate: fp8 DoubleRow
                            for g in range(PG):
                                nc.tensor.matmul(
                                    ps_p[:, :RN],
                                    lhsT=wih8_t[:, g, :, gu * P:(gu + 1) * P],
                                    rhs=xp8_t[:, g, :, :],
                                    start=(g == 0), stop=(g == PG - 1),
                                    perf_mode=DR)
                        else:
                            for c in range(CG):
                                nc.tensor.matmul(
                                    ps_p[:, :RN],
                                    lhsT=wih_t[:, c, (gu - UG) * P:(gu - UG + 1) * P],
                                    rhs=xp_t[:, c, :],
                                    start=(c == 0), stop=(c == CG - 1))
                        nc.vector.tensor_copy(gi_pre[:, gu, :], ps_p[:, :RN])

            if use_seed:
                emit_whh()  # pass 0 already needs W_hh

            h_cur = None   # fp16 SBUF (P, CG, m_j[j]) input h for current pass
            h8_cur = None  # fp8 SBUF (P, PG, 2, m_j[j]) for r-gate DoubleRow
            for j in range(Lmax):
                if j == j_pre and gi_pre is not None:
                    emit_presweep()
                scope = nc.named_scope(f"pass{j:02d}")
                scope.__enter__()
                m = m_j[j]
                m_next = m_j[j + 1] if j + 1 < Lmax else 0
                has_h = (j > 0) or use_seed
                pre = j >= j_pre
                base = int(M_off[j])
                h_next = (hpool.tile([P, CG, m_next], bf16, tag="hbuf",
                                     name=f"hbuf{j}")
                          if m_next > 0 else None)
                h8_next = (hpool.tile([P, PG, 2, m_next], fp8, tag="hbuf8",
                                      name=f"hbuf8_{j}")
                           if m_next > 0 else None)

                for ci, (off, f) in enumerate(_chunks(m)):
                    if not pre:
                        x_t = get_x_tile(j, off, f)
                        x8_t = get_x8_tile(j, off, f)
                    if j == 0 and use_seed:
                        hs_t = xpool.tile([P, CG, CH], bf16, tag="hseed", name="hs_t", bufs=1)
                        hs8_t = xpool.tile([P, PG, 2, CH], fp8, tag="hseed8",
                                           name="hs8_t", bufs=1)
                        for c in range(CG):
                            nc.sync.dma_start(
                                out=hs_t[:, c, :f],
                                in_=hseedT[c * P:(c + 1) * P, off: off + f])
                        for g in range(PG):
                            for i in range(2):
                                c = 2 * g + i
                                nc.sync.dma_start(
                                    out=hs8_t[:, g, i, :f],
                                    in_=hseed8T[c * P:(c + 1) * P, off: off + f])
                        h_in = lambda c: hs_t[:, c, :f]
                        h8_in = lambda g: hs8_t[:, g, :, :f]
                    elif has_h:
                        h_in = lambda c: h_cur[:, c, off: off + f]
                        h8_in = lambda g: h8_cur[:, g, :, off: off + f]
                    else:
                        h_in = h8_in = None
                    # presweep-relative row slice for this chunk
                    p0 = base + off - R0

                    def x_mms_r(ps, stop_at_end):
                        for g in range(PG):
                            nc.tensor.matmul(
                                ps[:, :f],
                                lhsT=wih8_t[:, g, :, u * P:(u + 1) * P],
                                rhs=x8_t[:, g, :, :f],
                                start=(g == 0),
                                stop=(stop_at_end and g == PG - 1),
                                perf_mode=DR)

                    def h_mms_r(ps, gs, do_start, do_stop):
                        gs = list(gs)
                        for g in gs:
                            nc.tensor.matmul(
                                ps[:, :f],
                                lhsT=whh8_t[:, g, :, u * P:(u + 1) * P],
                                rhs=h8_in(g),
                                start=(do_start and g == gs[0]),
                                stop=(do_stop and g == gs[-1]),
                                perf_mode=DR,
                                skip_group_check=True)

                    def x_mms(ps, gate, stop_at_end):
                        # gate: 0=z, 1=n in the fp16 weight tiles
                        for c in range(CG):
                            nc.tensor.matmul(
                                ps[:, :f],
                                lhsT=wih_t[:, c, gate * UNITS + u * P:
                                           gate * UNITS + (u + 1) * P],
                                rhs=x_t[:, c, :f],
                                start=(c == 0),
                                stop=(stop_at_end and c == CG - 1))

                    def h_mms(ps, gate, cs, do_start, do_stop):
                        cs = list(cs)
                        for c in cs:
                            nc.tensor.matmul(
                                ps[:, :f],
                                lhsT=whh_t[:, c, gate * UNITS + u * P:
                                           gate * UNITS + (u + 1) * P],
                                rhs=h_in(c),
                                start=(do_start and c == cs[0]),
                                stop=(do_stop and c == cs[-1]),
                                skip_group_check=True)

                    for u in range(UG):
                        ps_r = ppool.tile([P, CH], f32, tag="ps_r")
                        ps_z = ppool.tile([P, CH], f32, tag="ps_z")
                        if not pre:
                            ps_gin = ppool.tile([P, CH], f32, tag="ps_gin")
                        ps_ghn = (ppool.tile([P, CH], f32, tag="ps_ghn",
                                             name="ps_ghn")
                                  if has_h else None)

                        # For the first unit-tile of a chunk, defer every
                        # gate's last h-matmul to the end: it waits on the
                        # previous pass's last h cast, and deferring lets the
                        # other matmuls run during that wait.
                        split = has_h and u == 0 and off == 0
                        early = range(CG - 1) if split else range(CG)
                        early_g = range(PG - 1) if split else range(PG)
                        if not pre:
                            x_mms_r(ps_r, stop_at_end=not has_h)
                            if has_h:
                                h_mms_r(ps_r, early_g, False, not split)
                            x_mms(ps_z, 0, stop_at_end=not has_h)
                            if has_h:
                                h_mms(ps_z, 0, early, False, not split)
                            x_mms(ps_gin, 1, stop_at_end=True)
                            if has_h:
                                h_mms(ps_ghn, 1, early, True, not split)
                        else:
                            h_mms_r(ps_r, early_g, True, not split)
                            h_mms(ps_z, 0, early, True, not split)
                            h_mms(ps_ghn, 1, early, True, not split)
                        if split:
                            h_mms_r(ps_r, [PG - 1], False, True)
                            h_mms(ps_z, 0, [CG - 1], False, True)
                            h_mms(ps_ghn, 1, [CG - 1], False, True)

                        r_sb = spool.tile([P, CH], bf16, tag="r")
                        z_sb = spool.tile([P, CH], bf16, tag="z")
                        n_sb = spool.tile([P, CH], bf16, tag="n")
                        h_sb = spool.tile([P, CH], bf16, tag="r" if use_seed else "h",
                                          name="h_sb")
                        t2 = spool.tile([P, CH], bf16, tag="t2")
                        if pre:
                            # ps_r and gi_pre_r are both *W8SCALE; fold the
                            # rescale into the activation's scale.
                            nc.vector.tensor_add(r_sb[:, :f], ps_r[:, :f],
                                                 gi_pre[:, u, p0:p0 + f])
                            nc.scalar.activation(r_sb[:, :f], r_sb[:, :f], Sig,
                                                 bias=b_t[:, u, 0:1], scale=RS)
                            nc.vector.scalar_tensor_tensor(
                                z_sb[:, :f], ps_z[:, :f], b_t[:, u, 1:2],
                                gi_pre[:, UG + u, p0:p0 + f], op0=ADD, op1=ADD)
                            nc.scalar.activation(z_sb[:, :f], z_sb[:, :f], Sig)
                            nc.vector.scalar_tensor_tensor(
                                t2[:, :f], ps_ghn[:, :f], b_t[:, u, 3:4],
                                r_sb[:, :f], op0=ADD, op1=MULT)
                            arg = spool.tile([P, CH], bf16, tag="d", name="arg")
                            nc.vector.tensor_add(arg[:, :f], t2[:, :f],
                                                 gi_pre[:, 2 * UG + u, p0:p0 + f])
                            nc.scalar.activation(n_sb[:, :f], arg[:, :f], Tanh,
                                                 bias=b_t[:, u, 2:3])
                        else:
                            nc.scalar.activation(r_sb[:, :f], ps_r[:, :f], Sig,
                                                 bias=b_t[:, u, 0:1], scale=RS)
                            nc.scalar.activation(z_sb[:, :f], ps_z[:, :f], Sig,
                                                 bias=b_t[:, u, 1:2])
                            if has_h:
                                # t2 = (ps_ghn + b_hhn) * r
                                nc.vector.scalar_tensor_tensor(
                                    t2[:, :f], ps_ghn[:, :f], b_t[:, u, 3:4],
                                    r_sb[:, :f], op0=ADD, op1=MULT)
                                arg = spool.tile([P, CH], bf16, tag="d", name="arg")
                                nc.vector.tensor_add(arg[:, :f], t2[:, :f],
                                                     ps_gin[:, :f])
                                nc.scalar.activation(n_sb[:, :f], arg[:, :f],
                                                     Tanh, bias=b_t[:, u, 2:3])
                            else:
                                # t2 = r*b_hhn + ps_gin ; n = tanh(t2 + b_ihn)
                                nc.vector.scalar_tensor_tensor(
                                    t2[:, :f], r_sb[:, :f], b_t[:, u, 3:4],
                                    ps_gin[:, :f], op0=MULT, op1=ADD)
                                nc.scalar.activation(n_sb[:, :f], t2[:, :f],
                                                     Tanh, bias=b_t[:, u, 2:3])
                        if has_h:
                            # h = n + z*(h_prev - n)   (h_prev via fp16 tile)
                            d_sb = spool.tile([P, CH], bf16, tag="d")
                            nc.vector.tensor_sub(d_sb[:, :f], h_in(u), n_sb[:, :f])
                            zd = spool.tile([P, CH], bf16, tag="t2", name="zd")
                            nc.vector.tensor_mul(zd[:, :f], z_sb[:, :f], d_sb[:, :f])
                            nc.vector.tensor_add(h_sb[:, :f], n_sb[:, :f], zd[:, :f])
                        else:
                            # h = (1-z)*n = n - z*n
                            zd = spool.tile([P, CH], bf16, tag="t2", name="zd")
                            nc.vector.tensor_mul(zd[:, :f], z_sb[:, :f], n_sb[:, :f])
                            nc.vector.tensor_sub(h_sb[:, :f], n_sb[:, :f], zd[:, :f])

                        nc.sync.dma_start(
                            out=outT[u * P:(u + 1) * P, base + off: base + off + f],
                            in_=h_sb[:, :f])
                        pf = min(m_next - off, f)
                        if pf > 0:
                            nc.vector.tensor_copy(h_next[:, u, off: off + pf],
                                                  h_sb[:, :pf])
                            nc.vector.tensor_copy(
                                h8_next[:, u // 2, u % 2, off: off + pf],
                                h_sb[:, :pf])
                    if j == 0 and ci == 0 and not use_seed:
                        emit_whh()  # W_hh drains during pass-0 compute
                    if not pre and (j, off) in x_tiles:
                        del x_tiles[(j, off)]  # consumed; let the slot recycle
                        del x8_tiles[(j, off)]
                h_cur = h_next
                h8_cur = h8_next
                scope.__exit__(None, None, None)
    nc.compile()
    return nc


# ------------------------------------------------------------------- kernel

def kernel(x, h0, reset, W_ih, W_hh, b_ih, b_hh):
    global LAST_EXEC_NS
    x = np.asarray(x, np.float32)
    h0 = np.asarray(h0, np.float32)
    reset_sb = np.asarray(reset).reshape(SEQ, B).astype(bool)
    W_ih = np.asarray(W_ih, np.float32)
    W_hh = np.asarray(W_hh, np.float32)
    b_ih = np.asarray(b_ih, np.float32)
    b_hh = np.asarray(b_hh, np.float32)

    h0_any = bool(np.any(h0))
    m_j, plans = _build_plan(reset_sb, h0_any)
    N_pad = sum(m_j)

    b_sum = b_ih + b_hh
    biases = np.stack([b_sum[:UNITS], b_sum[UNITS:2 * UNITS],
                       b_ih[2 * UNITS:], b_hh[2 * UNITS:]], axis=1)
    biases = np.ascontiguousarray(biases, np.float32)
    e4m3 = ml_dtypes.float8_e4m3  # TRN FP8_EXP4 (max +-240)
    # fp16 weights: z and n gates only; r gate is fp8 e4m3 scaled by W8SCALE
    wihT = np.ascontiguousarray(W_ih[UNITS:].T).astype(np.float16)
    whhT = np.ascontiguousarray(W_hh[UNITS:].T).astype(np.float16)
    wih8T = np.ascontiguousarray((W_ih[:UNITS] * W8SCALE).T).astype(e4m3)
    whh8T = np.ascontiguousarray((W_hh[:UNITS] * W8SCALE).T).astype(e4m3)

    xf = x.reshape(SEQ * B, DIM)
    in_maps = []
    for c in range(NCORES):
        tok, seed_b = plans[c]
        real = tok >= 0
        xg = np.zeros((N_pad, DIM), np.float32)
        xg[real] = xf[tok[real]]
        xgT = np.ascontiguousarray(xg.T)
        m = {
            "xT": xgT.astype(np.float16),
            "xT8": np.clip(xgT, -240, 240).astype(e4m3),
            "wihT": wihT, "whhT": whhT,
            "wih8T": wih8T, "whh8T": whh8T, "biases": biases,
        }
        if h0_any:
            hs = np.zeros((m_j[0], UNITS), np.float32)
            sreal = seed_b >= 0
            hs[sreal] = h0[seed_b[sreal]]
            hsT = np.ascontiguousarray(hs.T)
            m["hseedT"] = hsT.astype(np.float16)
            m["hseed8T"] = np.clip(hsT, -240, 240).astype(e4m3)
        in_maps.append(m)

    j_pre = 1
    while j_pre < len(m_j) and sum(m_j[j_pre:]) > CH:
        j_pre += 1
    nc = _build_nc(m_j, use_seed=h0_any, j_pre=j_pre)
    trace = os.environ.get("GRU_TRACE", "0") == "1"
    res = run_bass_kernel_spmd(nc, in_maps, list(range(NCORES)), trace=trace)
    LAST_EXEC_NS = res.exec_time_ns

    out = np.zeros((SEQ * B, UNITS), np.float32)
    for c in range(NCORES):
        tok, _ = plans[c]
        real = tok >= 0
        out[tok[real]] = res.results[c]["outT"].T[real]
    return out.reshape(SEQ, B, UNITS)

